# revision 27
# baseline (speedup 1.0000x reference)
"""Segment mean-pool (BERT lattice embedding) Trainium2 Bass kernel.

Full-input contract: kernel(hidden[64,512,768] f32, word_ids[64,512] i32,
num_tokens=400) -> [64,400,768] f32.

Strategy: data-parallel over batch across 8 NeuronCores (8 samples each).
Per sample b the ragged segment mean  out[t] = mean_{s: wid[s]==t} hidden[s]
is computed as a matmul on the PE array:

    A_T[s, t] = (word_ids[b, s] == t)            one-hot, built on-device
    psum[t, :] = sum_j A_T[j-chunk].T @ hidden[b, j-chunk]
    out[t, h] = psum[t, h] * recip[b, t]         recip = 1/max(count,1)

All matmuls run in float32r (FP22-truncated fp32): full PE rate at even
N>=256, ~2e-4 relative error, and no dtype casts of the 100 MB activation
tensor.  (fp16/bf16 would halve the LDWEIGHTS time that paces the PE,
but the required f32->16-bit casts are ~28 us of ACT/DVE work that
starves the PSUM->SBUF->DMA drain those engines also carry — measured
net loss every time.  SWDGE can cast inside the DMA but its Q7
descriptor generator is ~7x too slow for this stream.)

The per-word piece counts (reciprocals) are derived on host from the
128 KB word_ids index tensor — index-side preprocessing, like the shard
layout transform; all heavy data stays on device.

Layouts are chosen for contiguous DMA descriptors and a cheap PE mix:
  - pieces:  partition p holds s = 128j+p -> 3 KB/partition descriptors
    (segment-sum is invariant to how s is split into K-chunks)
  - words:   M-chunks {128,128,128,16}: the 16-wide runt's LDWEIGHTS is
    ~2x cheaper, and the LAST output write per sample is tiny, so the
    drain tail is short.

DMA plan (kernel is HBM-bound: 12.6 MB in + 9.8 MB out per core at a
~415 GB/s practical per-core ceiling = ~54 us of unavoidable streaming):
  - one merged aux tensor (word ids + reciprocals, 256 B/partition) at
    the head of the sync ring — NOT two tiny-packet transfers;
  - all hidden prefetches on the sync HWDGE ring (sample 0 split per
    j-chunk so the first accumulation starts as soon as chunk 0 lands);
  - ALL output DMAs go on the sync ring, BEHIND the inputs: ring FIFO
    guarantees the input stream runs solo at ~410 GB/s (done by ~40 us,
    so the PE is never input-starved — outputs sharing HBM mid-phase
    measurably starves the PE for ~11 us around samples 4-5), while
    scaled chunks pile up in a deep om buffer (~20 x 3 KB/partition)
    and then drain at ring max.  Total = input-solo + output-drain
    lands within ~1 us of the HBM floor, which interleaving cannot
    beat anyway.
"""

import numpy as np

B, S, H, T = 64, 512, 768, 400
N_CORES = 8
B_LOC = B // N_CORES  # samples per core
P = 128
J = S // P  # contraction chunks per sample
N0 = 384  # h-chunk split: two equal psum banks, balances the scale engines
M_CHUNKS = [(0, 128), (128, 128), (256, 128), (384, T - 384)]  # (t0, mw)
NM = len(M_CHUNKS)

_CACHED = {}


def build_program():
    """Build + compile the single-core Bass program (same NEFF on all cores)."""
    import concourse.bass as bass  # noqa: F401
    import concourse.mybir as mybir
    import concourse.tile as tile
    from concourse import bacc

    nc = bacc.Bacc(
        "TRN2",
        target_bir_lowering=False,
        debug=False,
        enable_asserts=False,
        num_devices=N_CORES,
    )
    f32 = mybir.dt.float32
    f32r = mybir.dt.float32r

    hidden_t = nc.dram_tensor("hidden", [B_LOC, S, H], f32r, kind="ExternalInput").ap()
    # aux[p, b, 0:4] = word_ids[b, 128j+p] (fp32; values < 400 exact), the
    # per-partition scalar for piece-chunk j.  aux[p, b, 4:8] =
    # 1/max(count,1) for word t = 128m + p (t >= 400 padded with 1.0).
    aux_t = nc.dram_tensor("aux_pb", [P, B_LOC, 2 * NM], f32, kind="ExternalInput").ap()
    out_t = nc.dram_tensor("out", [B_LOC, T, H], f32, kind="ExternalOutput").ap()

    with tile.TileContext(nc) as tc:
        with tc.tile_pool(name="const", bufs=1) as const_pool, \
             tc.tile_pool(name="hidp", bufs=B_LOC) as hid_pool, \
             tc.tile_pool(name="aTp", bufs=4) as aT_pool, \
             tc.tile_pool(name="outp", bufs=20) as out_pool, \
             tc.tile_pool(name="psum", bufs=4, space="PSUM") as psum_pool:

            aux_sb = const_pool.tile([P, B_LOC, 2 * NM], f32, name="aux_sb")
            nc.sync.dma_start(out=aux_sb, in_=aux_t)

            iota_t = const_pool.tile([P, T], f32, name="iota_t")
            nc.gpsimd.iota(
                iota_t,
                pattern=[[1, T]],
                base=0,
                channel_multiplier=0,
                allow_small_or_imprecise_dtypes=True,
            )

            # Prefetch the whole input shard up front (fits in SBUF): the
            # input queue streams back-to-back from t=0 and compute is never
            # input-starved. One DMA per sample; 3 KB descriptors measured
            # faster end-to-end than 12 KB ones.
            hids = []
            for b in range(B_LOC):
                hid = hid_pool.tile([P, J, H], f32r, name=f"hid{b}", tag="hid")
                src = hidden_t[b].rearrange("(j p) h -> p j h", p=P)
                if b == 0:
                    # First sample split per j-chunk so the first accumulation
                    # can start as soon as chunk 0 lands.
                    for j in range(J):
                        nc.sync.dma_start(out=hid[:, j, :], in_=src[:, j, :])
                else:
                    nc.sync.dma_start(out=hid, in_=src)
                hids.append(hid)

            for b in range(B_LOC):
                hid = hids[b]
                aT = aT_pool.tile([P, J, T], f32r, name="aT", tag="aT")
                for j in range(J):
                    nc.vector.tensor_scalar(
                        aT[:, j, :],
                        iota_t,
                        aux_sb[:, b, j : j + 1],
                        None,
                        op0=mybir.AluOpType.is_equal,
                    )
                for mi, (t0, mw) in enumerate(M_CHUNKS):
                    ps0 = psum_pool.tile([P, N0], f32, name="ps0", tag="ps0")
                    ps1 = psum_pool.tile([P, H - N0], f32, name="ps1", tag="ps1")
                    for j in range(J):
                        nc.tensor.matmul(
                            ps0[:mw],
                            aT[:, j, t0 : t0 + mw],
                            hid[:, j, 0:N0],
                            start=(j == 0),
                            stop=(j == J - 1),
                        )
                    for j in range(J):
                        nc.tensor.matmul(
                            ps1[:mw],
                            aT[:, j, t0 : t0 + mw],
                            hid[:, j, N0:H],
                            start=(j == 0),
                            stop=(j == J - 1),
                        )

                    rec = aux_sb[:, b, NM + mi : NM + mi + 1]
                    om = out_pool.tile([P, H], f32, name="om", tag="om")
                    # out = psum * (1/count): ACT and DVE each take one chunk,
                    # both read PSUM directly.
                    nc.scalar.mul(om[:mw, 0:N0], ps0[:mw], rec[:mw])
                    nc.vector.tensor_scalar_mul(om[:mw, N0:H], ps1[:mw], rec[:mw])
                    # Per-m-chunk output DMA, issued as soon as its scale is
                    # done — data flows once the sync ring finishes the
                    # input prefetch.  (Issuing outputs from the ACT
                    # sequencer instead — even just the last two samples' —
                    # measurably stretches the whole scale/PSUM-recycle
                    # cadence, and a SWDGE tail for the last sample measured
                    # neutral-to-worse; the sync sequencer is idle and free.)
                    nc.sync.dma_start(out=out_t[b, t0 : t0 + mw], in_=om[:mw])

    nc.compile()
    return nc


def _prep_in_maps(hidden, word_ids):
    hidden = np.ascontiguousarray(np.asarray(hidden), dtype=np.float32).reshape(B, S, H)
    wid = np.ascontiguousarray(np.asarray(word_ids), dtype=np.int32).reshape(B, S)

    # Per-word piece counts -> 1/max(count,1), padded to 512 words.
    counts = np.zeros((B, P * NM), np.int64)
    rows = np.repeat(np.arange(B), S)
    np.add.at(counts, (rows, wid.reshape(-1)), 1)
    recip = (1.0 / np.maximum(counts, 1)).astype(np.float32)  # [B, 512]

    in_maps = []
    for i in range(N_CORES):
        sl = slice(i * B_LOC, (i + 1) * B_LOC)
        hs = np.ascontiguousarray(hidden[sl])
        ws = wid[sl]
        aux = np.ones((P, B_LOC, 2 * NM), np.float32)
        # aux[p, b, j] = wid[b, 128j+p]
        aux[:, :, :NM] = ws.reshape(B_LOC, J, P).transpose(2, 0, 1)
        # aux[p, b, 4+m] = recip[b, 128m+p]
        aux[:, :, NM:] = recip[sl].reshape(B_LOC, NM, P).transpose(2, 0, 1)
        in_maps.append({"hidden": hs, "aux_pb": np.ascontiguousarray(aux)})
    return in_maps


def run(hidden, word_ids, trace=False, **trace_kwargs):
    from concourse import bass_utils

    if "nc" not in _CACHED:
        _CACHED["nc"] = build_program()
    nc = _CACHED["nc"]
    in_maps = _prep_in_maps(hidden, word_ids)
    res = bass_utils.run_bass_kernel_spmd(
        nc, in_maps, core_ids=list(range(N_CORES)), trace=trace, **trace_kwargs
    )
    out = np.concatenate([res.results[i]["out"] for i in range(N_CORES)], axis=0)
    return out.astype(np.float32, copy=False), res


def kernel(hidden, word_ids, num_tokens=None, **_unused):
    out, _ = run(hidden, word_ids, trace=False)
    return out
